# revision 9
# baseline (speedup 1.0000x reference)
"""Causal self-attention (B=4, T=2048, C=1024, 16 heads x 64) on 8 TRN2 NeuronCores.

Sharding: tensor-parallel over heads. Core c owns heads {2c, 2c+1}:
  - w_attn column slices -> per-core QKV in transposed layout (dims on
    partitions, tokens on free dim),
  - attention in S^T form: S^T[k,q] = matmul(lhsT=kT, rhs=qT_headzeroed),
    softmax denominator via ones-columns appended to V, PV consumes exp(S^T)
    directly (no transposes anywhere in the attention inner loop),
  - partial output projection in transposed layout (out dims on partitions,
    so b_proj is a per-partition bias),
  - host sums the 8 partial projections (the TP all-reduce).

All matmuls run in bf16 (1 cycle/row on the PE vs ~2 for f32r, and half-cost
LDWEIGHTS).  PSUM accumulation stays f32.  The output partials are written
bf16 and accumulated f32 on the host.

Per-chunk softmax normalization avoids the PE entirely: reciprocal of the
denominator row is DMA-broadcast across partitions, so the PE stream never
waits on the normalize chain (keeps the PE p-state at max clock).
"""

import sys
import numpy as np

sys.path.insert(0, "/opt/trn_rl_repo")

B, T, C = 4, 2048, 1024
NH, HD = 16, 64
NCORES = 8
TOK = B * T                 # 8192 tokens
NCH = TOK // 512            # 16 token chunks of 512
CHB = T // 512              # 4 chunks per batch
NKB_B = T // 128            # 16 k-blocks per batch
SCALE = 1.0 / 8.0

_CACHE = {}


def _build_program():
    import concourse.tile as tile
    from concourse import bacc, mybir
    from concourse.masks import make_identity

    f32 = mybir.dt.float32
    bf16 = mybir.dt.bfloat16

    nc = bacc.Bacc("TRN2", target_bir_lowering=False, debug=False,
                   num_devices=NCORES)

    xT = nc.dram_tensor("xT", [C, TOK], bf16, kind="ExternalInput").ap()
    wqkv = nc.dram_tensor("wqkv", [C, 384], bf16, kind="ExternalInput").ap()
    battn = nc.dram_tensor("battn", [128, 3], f32, kind="ExternalInput").ap()
    wproj = nc.dram_tensor("wproj", [128, C], bf16, kind="ExternalInput").ap()
    bproj = nc.dram_tensor("bproj", [128, 8], f32, kind="ExternalInput").ap()
    outT = nc.dram_tensor("outT", [C, TOK], bf16, kind="ExternalOutput").ap()

    with tile.TileContext(nc) as tc:
        with nc.allow_low_precision(reason="bf16 attention within 2e-2 tol"), \
             tc.tile_pool(name="const", bufs=1) as const, \
             tc.tile_pool(name="resid", bufs=1) as resid:
            # constants: identity | mask (bf16), battn | bproj (f32)
            combo = const.tile([128, 256], bf16, tag="combo")
            ident = combo[:, 0:128]
            mask = combo[:, 128:256]
            make_identity(nc, ident)
            # mask[k, q] = 1.0 where k <= q else 0 (upper triangular incl diag)
            nc.gpsimd.memset(mask, 0.0)
            nc.gpsimd.affine_select(
                out=mask, in_=mask,
                compare_op=mybir.AluOpType.is_gt,
                fill=1.0, base=0, pattern=[[-1, 128]], channel_multiplier=1,
            )
            bias_sb = const.tile([128, 11], f32, tag="bias")
            battn_sb = bias_sb[:, 0:3]
            bp_sb = bias_sb[:, 3:11]
            nc.sync.dma_start(battn_sb, battn[:])
            nc.sync.dma_start(bp_sb, bproj[:])
            wp_sb = const.tile([128, C], bf16, tag="wp")
            nc.sync.dma_start(wp_sb[:], wproj[:])

            # resident activations (bf16, matmul operands at base partition 0)
            kT = resid.tile([128, NCH, 512], bf16, tag="kT")
            qz0 = resid.tile([128, NCH, 512], bf16, tag="qz0")
            qz1 = resid.tile([128, NCH, 512], bf16, tag="qz1")
            vpr = resid.tile([128, NKB_B * B, 132], bf16, tag="vpr")

            nc.vector.memset(qz0[64:128, :, :], 0.0)
            nc.gpsimd.memset(qz1[0:64, :, :], 0.0)
            nc.gpsimd.memset(vpr[:, :, 64:66], 1.0)
            nc.gpsimd.memset(vpr[:, :, 130:132], 1.0)

            # ---------------- Phase A: QKV + V transposes ----------------
            with tc.tile_pool(name="phA", bufs=1) as phA, \
                 tc.tile_pool(name="xin", bufs=6) as xin, \
                 tc.tile_pool(name="vsb", bufs=2) as vsb, \
                 tc.tile_pool(name="psA", bufs=3, space="PSUM") as psA, \
                 tc.tile_pool(name="psT", bufs=2, space="PSUM") as psT:
                w_sb = phA.tile([128, 8, 384], bf16, tag="w_sb")
                nc.sync.dma_start(w_sb[:], wqkv.rearrange("(ko p) m -> p ko m", p=128))
                xTr = xT.rearrange("(ko p) t -> p ko t", p=128)
                for ch in range(NCH):
                    xa = xin.tile([128, 8, 512], bf16, tag="xc")
                    nc.sync.dma_start(xa[:], xTr[:, :, ch * 512:(ch + 1) * 512])
                    for m in range(3):
                        ps = psA.tile([128, 512], f32, tag="m")
                        for ko in range(8):
                            nc.tensor.matmul(ps[:], w_sb[:, ko, m * 128:(m + 1) * 128],
                                             xa[:, ko, :], start=(ko == 0), stop=(ko == 7))
                        if m == 0:
                            nc.vector.tensor_scalar_add(qz0[0:64, ch, :], ps[0:64, :],
                                                        battn_sb[0:64, 0:1])
                            nc.vector.tensor_scalar_add(qz1[64:128, ch, :], ps[64:128, :],
                                                        battn_sb[64:128, 0:1])
                        elif m == 1:
                            nc.scalar.activation(kT[:, ch, :], ps[:],
                                                 mybir.ActivationFunctionType.Identity,
                                                 bias=battn_sb[:, 1:2])
                        else:
                            vs = vsb.tile([128, 512], bf16, tag="vs")
                            nc.vector.tensor_scalar_add(vs[:], ps[:], battn_sb[:, 2:3])
                            for t in range(4):
                                pst = psT.tile([128, 128], bf16, tag="tp")
                                nc.tensor.transpose(pst[:], vs[:, t * 128:(t + 1) * 128], ident)
                                gkb = ch * 4 + t
                                nc.vector.tensor_copy(vpr[:, gkb, 0:64], pst[:, 0:64])
                                nc.vector.tensor_copy(vpr[:, gkb, 66:130], pst[:, 64:128])

            # ---------------- Phase B: attention + inline projection ----------------
            with tc.tile_pool(name="pp", bufs=3) as ppool, \
                 tc.tile_pool(name="nrm", bufs=2) as nrm, \
                 tc.tile_pool(name="sby", bufs=2) as sbyp, \
                 tc.tile_pool(name="ytc", bufs=3) as ytc, \
                 tc.tile_pool(name="ob", bufs=3) as obp, \
                 tc.tile_pool(name="psB", bufs=2, space="PSUM") as psB:

                def emit_proj_pair(qch, yTch, p):
                    # projection od pair {2p, 2p+1} of the PREVIOUS chunk;
                    # interleaved into the current chunk's S/PV stream so the
                    # PE never idles while normalize chains run
                    psP = psB.tile([128, 2, 512], f32, tag="s")
                    for h in range(2):
                        od = 2 * p + h
                        nc.tensor.matmul(psP[:, h, :], wp_sb[:, od * 128:(od + 1) * 128],
                                         yTch[:], start=True, stop=True)
                    oSb = obp.tile([128, 2, 512], bf16, tag="o")
                    nc.vector.tensor_scalar_add(oSb[:, 0, :], psP[:, 0, :],
                                                bp_sb[:, 2 * p:2 * p + 1])
                    nc.scalar.activation(oSb[:, 1, :], psP[:, 1, :],
                                         mybir.ActivationFunctionType.Identity,
                                         bias=bp_sb[:, 2 * p + 1:2 * p + 2])
                    for h in range(2):
                        od = 2 * p + h
                        nc.sync.dma_start(
                            outT[od * 128:(od + 1) * 128, qch * 512:(qch + 1) * 512],
                            oSb[:, h, :])

                pending = []
                for b in range(B):
                    for j in range(CHB):
                        qch = b * CHB + j
                        psY = psB.tile([128, 2, 512], f32, tag="y")
                        nkb = 4 * j + 4

                        def emit_S(kb):
                            vstart = max(0, kb * 128 - j * 512)
                            kch = b * CHB + kb // 4
                            ksub = (kb % 4) * 128
                            psS = psB.tile([128, 2, 512], f32, tag="s")
                            nc.tensor.matmul(psS[:, 0, vstart:], kT[:, kch, ksub:ksub + 128],
                                             qz0[:, qch, vstart:], start=True, stop=True)
                            nc.tensor.matmul(psS[:, 1, vstart:], kT[:, kch, ksub:ksub + 128],
                                             qz1[:, qch, vstart:], start=True, stop=True)
                            return psS, vstart

                        # software-pipelined: S(kb+1) is emitted ahead of PV(kb)
                        # so the in-order PE never waits on exp(kb)
                        prev = emit_S(0)
                        for kb in range(nkb):
                            psS, vstart = prev
                            Pb = ppool.tile([128, 2, 512], bf16, tag="p")
                            nc.scalar.activation(Pb[:, :, vstart:], psS[:, :, vstart:],
                                                 mybir.ActivationFunctionType.Exp, scale=SCALE)
                            if kb + 1 < nkb:
                                prev = emit_S(kb + 1)
                            if len(pending) >= 2 and kb < 4:
                                emit_proj_pair(*pending[0], kb)
                            if kb >= 4 * j:
                                nc.vector.tensor_mul(
                                    Pb[:, :, vstart:vstart + 128],
                                    Pb[:, :, vstart:vstart + 128],
                                    mask[:, None, :].to_broadcast((128, 2, 128)))
                            gkb = b * NKB_B + kb
                            nc.tensor.matmul(psY[0:66, 0, vstart:], vpr[:, gkb, 0:66],
                                             Pb[:, 0, vstart:], start=(kb == 0), stop=(kb == nkb - 1))
                            nc.tensor.matmul(psY[0:66, 1, vstart:], vpr[:, gkb, 66:132],
                                             Pb[:, 1, vstart:], start=(kb == 0), stop=(kb == nkb - 1))
                        if len(pending) >= 2:
                            pending.pop(0)
                        # normalize: spread the 1-partition denom row across 128
                        # partitions, reciprocal there (128 lanes, fast), gather
                        # back, gpsimd-broadcast. The PE never waits on this
                        # chain: the projection consuming yTch runs two chunks
                        # later.
                        sbY = sbyp.tile([66, 2, 512], f32, tag="sby")
                        nc.vector.tensor_copy(sbY[:], psY[0:66, :, :])
                        sc = nrm.tile([128, 8], f32, tag="sc")
                        nc.sync.dma_start(sc[:], sbY[64:65, :, :])
                        scR = nrm.tile([128, 8], bf16, tag="scR")
                        nc.vector.reciprocal(scR[:], sc[:])
                        rr = nrm.tile([1, 1024], bf16, tag="rr")
                        nc.sync.dma_start(rr[0:1, :], scR[:])
                        rB = nrm.tile([64, 2, 512], bf16, tag="rB")
                        nc.gpsimd.partition_broadcast(rB[:], rr[0:1, :])
                        yTch = ytc.tile([128, 512], bf16, tag="yt")
                        nc.vector.tensor_mul(yTch[0:64, :], sbY[0:64, 0, :], rB[:, 0, :])
                        yst = nrm.tile([64, 512], bf16, tag="yst")
                        nc.vector.tensor_mul(yst[:], sbY[0:64, 1, :], rB[:, 1, :])
                        nc.sync.dma_start(yTch[64:128, :], yst[:])
                        pending.append((qch, yTch))
                # drain the last chunk's projection
                for qch, yTch in pending:
                    for p in range(4):
                        emit_proj_pair(qch, yTch, p)

    nc.compile()
    return nc


def _get_program():
    if "nc" not in _CACHE:
        _CACHE["nc"] = _build_program()
    return _CACHE["nc"]


def kernel(x, w_attn, b_attn, w_proj, b_proj, _trace=False):
    import ml_dtypes
    from concourse.bass_utils import run_bass_kernel_spmd

    nc = _get_program()
    bf = ml_dtypes.bfloat16

    x = np.asarray(x, dtype=np.float32)
    w_attn = np.asarray(w_attn, dtype=np.float32)
    b_attn = np.asarray(b_attn, dtype=np.float32)
    w_proj = np.asarray(w_proj, dtype=np.float32)
    b_proj = np.asarray(b_proj, dtype=np.float32)

    xT_np = np.ascontiguousarray(x.reshape(TOK, C).T.astype(bf))

    in_maps = []
    for c in range(NCORES):
        lo, hi = c * 128, (c + 1) * 128
        wq = w_attn[:, lo:hi]
        wk = w_attn[:, C + lo:C + hi]
        wv = w_attn[:, 2 * C + lo:2 * C + hi]
        wqkv_np = np.ascontiguousarray(
            np.concatenate([wq, wk, wv], axis=1).astype(bf))
        bq = b_attn[lo:hi]
        bk = b_attn[C + lo:C + hi]
        bv = b_attn[2 * C + lo:2 * C + hi]
        battn_np = np.ascontiguousarray(np.stack([bq, bk, bv], axis=1))  # [128, 3]
        wproj_np = np.ascontiguousarray(w_proj[lo:hi, :].astype(bf))
        if c == 0:
            bproj_np = np.ascontiguousarray(b_proj.reshape(8, 128).T)
        else:
            bproj_np = np.zeros((128, 8), dtype=np.float32)
        in_maps.append({
            "xT": xT_np,
            "wqkv": wqkv_np,
            "battn": battn_np,
            "wproj": wproj_np,
            "bproj": bproj_np,
        })

    res = run_bass_kernel_spmd(nc, in_maps, core_ids=list(range(NCORES)),
                               trace=_trace)
    acc = res.results[0]["outT"].astype(np.float32)
    for c in range(1, NCORES):
        acc += res.results[c]["outT"].astype(np.float32)
    out = np.ascontiguousarray(acc.T).reshape(B, T, C)
    if _trace:
        kernel.last_exec_time_ns = res.exec_time_ns
        kernel.last_scope_times = res.per_core_scope_times
        kernel.last_trace = res.instructions_and_trace
    return out
